# revision 3
# baseline (speedup 1.0000x reference)
"""GraphVAE (4x GAT + decoder) Trainium2 kernel, 8 NeuronCores.

Destination-node sharding: host bin-packs 50000 nodes into 392 windows of
128 slots so each window's in-edges (incl self-loops) fit 9x128 chunks.
Core c owns windows [49c, 49c+49). Per chunk: indirect-DMA row gather of
source features, is_equal one-hot, PE matmul scatter-add into per-window
PSUM, post-divide softmax. Halo exchange: AllGather of projected tables.
"""
import heapq
import numpy as np

import concourse.bass as bass
import concourse.bacc as bacc
import concourse.mybir as mybir
import concourse.tile as tile
from concourse.bass_utils import run_bass_kernel_spmd

F32 = mybir.dt.float32
I32 = mybir.dt.int32

N = 50000
E = 400000
IN = 64
LAT = 32
NEG = 0.2

NC = 8
SH = 6272
NP = NC * SH
W = 49
CH = 9
CAP = CH * 128
NWIN = NC * W
NCHUNK = W * CH


def _binpack(dst_all):
    deg = np.bincount(dst_all, minlength=N)
    order = np.argsort(-deg, kind="stable")
    heap = [(0, 0, w) for w in range(NWIN)]
    heapq.heapify(heap)
    win_of = np.empty(N, np.int32)
    slot_of = np.empty(N, np.int32)
    pending = []
    for v in order:
        d = int(deg[v])
        while True:
            load, cnt, w = heapq.heappop(heap)
            if cnt < 128 and load + d <= CAP:
                win_of[v] = w
                slot_of[v] = cnt
                heapq.heappush(heap, (load + d, cnt + 1, w))
                break
            pending.append((load, cnt, w))
            if not heap:
                raise RuntimeError("binpack failed")
        for it in pending:
            heapq.heappush(heap, it)
        pending.clear()
    return win_of, slot_of


def _prep(edge_index):
    src0 = np.asarray(edge_index[0], np.int64)
    dst0 = np.asarray(edge_index[1], np.int64)
    loops = np.arange(N, dtype=np.int64)
    src_all = np.concatenate([src0, loops])
    dst_all = np.concatenate([dst0, loops])

    win_of, slot_of = _binpack(dst_all)
    pnode = win_of * 128 + slot_of
    inv = np.zeros(NP, np.int64)
    inv[pnode] = np.arange(N)
    used = np.zeros(NP, bool)
    used[pnode] = True

    ew = win_of[dst_all]
    order = np.argsort(ew, kind="stable")
    counts = np.bincount(ew, minlength=NWIN)
    offs = np.zeros(NWIN + 1, np.int64)
    np.cumsum(counts, out=offs[1:])

    idx_src = np.zeros((NC, 128, NCHUNK), np.int32)
    idx_dstl = np.zeros((NC, 128, NCHUNK), np.int32)
    dstl_f = np.full((NC, 128, NCHUNK), -1.0, np.float32)
    epos = np.full((NC, 128, NCHUNK), -1, np.int64)

    for c in range(NC):
        for wl in range(W):
            w = c * W + wl
            el = order[offs[w]:offs[w + 1]]
            ns = len(el)
            assert ns <= CAP
            s = pnode[src_all[el]]
            dl = slot_of[dst_all[el]]
            dloc = (win_of[dst_all[el]] - c * W) * 128 + dl
            pos = np.where(el < E, el, -1)
            for j in range(CH):
                lo, hi = j * 128, min(j * 128 + 128, ns)
                n = max(0, hi - lo)
                col = wl * CH + j
                if n > 0:
                    idx_src[c, :n, col] = s[lo:hi]
                    idx_dstl[c, :n, col] = dloc[lo:hi]
                    dstl_f[c, :n, col] = dl[lo:hi]
                    epos[c, :n, col] = pos[lo:hi]
    return dict(pnode=pnode, inv=inv, used=used, idx_src=idx_src,
                idx_dstl=idx_dstl, dstl_f=dstl_f, epos=epos)


def _fold(Wm, a_src, a_dst):
    K = Wm.shape[0]
    h, C = a_src.shape
    Wr = Wm.reshape(K, h, C)
    A = np.einsum("khc,hc->kh", Wr, a_src)
    B = np.einsum("khc,hc->kh", Wr, a_dst)
    return np.concatenate([Wm, A, B], axis=1).astype(np.float32)


def _build(b_edge_val):
    import contextlib
    nc = bacc.Bacc("TRN2", target_bir_lowering=False, num_devices=NC)
    dt = F32

    xT = nc.dram_tensor("xT", [IN, NP], dt, kind="ExternalInput")
    W1p = nc.dram_tensor("W1p", [IN, 264], dt, kind="ExternalInput")
    W2p = nc.dram_tensor("W2p", [256, 264], dt, kind="ExternalInput")
    Wml = nc.dram_tensor("Wml", [256, 68], dt, kind="ExternalInput")
    Wfc1 = nc.dram_tensor("Wfc1", [33, 64], dt, kind="ExternalInput")
    Wnode1 = nc.dram_tensor("Wnode1", [65, 64], dt, kind="ExternalInput")
    b1b = nc.dram_tensor("b1b", [128, 256], dt, kind="ExternalInput")
    b2b = nc.dram_tensor("b2b", [128, 256], dt, kind="ExternalInput")
    bmlb = nc.dram_tensor("bmlb", [128, 64], dt, kind="ExternalInput")
    wedgeb = nc.dram_tensor("wedgeb", [128, 64], dt, kind="ExternalInput")
    iota = nc.dram_tensor("iota", [128, 128], dt, kind="ExternalInput")
    ident = nc.dram_tensor("ident", [128, 128], dt, kind="ExternalInput")
    isrc = nc.dram_tensor("isrc", [128, NCHUNK], I32, kind="ExternalInput")
    idstl = nc.dram_tensor("idstl", [128, NCHUNK], I32, kind="ExternalInput")
    dstlf = nc.dram_tensor("dstlf", [128, NCHUNK], dt, kind="ExternalInput")
    iselft = nc.dram_tensor("iselft", [128, W], I32, kind="ExternalInput")
    epst = nc.dram_tensor("epst", [128, W * LAT], dt, kind="ExternalInput")

    mu_o = nc.dram_tensor("mu_o", [128, W * LAT], dt, kind="ExternalOutput")
    lv_o = nc.dram_tensor("lv_o", [128, W * LAT], dt, kind="ExternalOutput")
    z_o = nc.dram_tensor("z_o", [128, W * LAT], dt, kind="ExternalOutput")
    nf_o = nc.dram_tensor("nf_o", [128, W * IN], dt, kind="ExternalOutput")
    sc_o = nc.dram_tensor("sc_o", [128, NCHUNK], dt, kind="ExternalOutput")

    T1 = nc.dram_tensor("T1", [NP, 264], dt)
    sh2 = nc.dram_tensor("sh2", [SH, 264], dt)
    T2 = nc.dram_tensor("T2", [NP, 264], dt, addr_space="Shared")
    shml = nc.dram_tensor("shml", [SH, 68], dt)
    Tml = nc.dram_tensor("Tml", [NP, 68], dt, addr_space="Shared")
    shu = nc.dram_tensor("shu", [SH, 64], dt)
    Tu = nc.dram_tensor("Tu", [NP, 64], dt, addr_space="Shared")
    Tzd = nc.dram_tensor("Tzd", [SH, 64], dt)

    stack = contextlib.ExitStack()
    iota_sb = stack.enter_context(nc.sbuf_tensor("iota_sb", [128, 128], dt))
    ident_sb = stack.enter_context(nc.sbuf_tensor("ident_sb", [128, 128], dt))
    isrc_sb = stack.enter_context(nc.sbuf_tensor("isrc_sb", [128, NCHUNK], I32))
    idstl_sb = stack.enter_context(nc.sbuf_tensor("idstl_sb", [128, NCHUNK], I32))
    dstlf_sb = stack.enter_context(nc.sbuf_tensor("dstlf_sb", [128, NCHUNK], dt))
    iself_sb = stack.enter_context(nc.sbuf_tensor("iself_sb", [128, W], I32))
    oT = stack.enter_context(nc.sbuf_tensor("oT", [128, 2 * SH], dt))
    sdst_sb = stack.enter_context(nc.sbuf_tensor("sdst_sb", [128, W * 4], dt))
    zT_sb = stack.enter_context(nc.sbuf_tensor("zT_sb", [64, SH], dt))
    zdT_sb = stack.enter_context(nc.sbuf_tensor("zdT_sb", [96, SH], dt))

    RG = [list(range(NC))]

    def agg_layer(pool, pacc, ptmp, table, sdw, nheads, chs, bias_sb, relu, out_cb):
        hw = chs * nheads
        ncol = table.shape[1]
        for w in range(W):
            G = pool.tile([128, CH, ncol], dt, tag="G")
            A = pool.tile([128, CH, 128], dt, tag="A")
            Bs = pool.tile([128, CH, 128], dt, tag="Bs")
            sc = pool.tile([128, CH * nheads], dt, tag="sc")
            for j in range(CH):
                col = w * CH + j
                nc.gpsimd.indirect_dma_start(
                    out=G[:, j, :], out_offset=None, in_=table[:],
                    in_offset=bass.IndirectOffsetOnAxis(ap=isrc_sb[:, col:col + 1], axis=0))
                nc.vector.tensor_tensor(
                    out=A[:, j, :], in0=dstlf_sb[:, col:col + 1].to_broadcast([128, 128]),
                    in1=iota_sb[:], op=mybir.AluOpType.is_equal)
                Bp = ptmp.tile([128, 128], dt, space="PSUM", tag="Bp")
                nc.tensor.transpose(out=Bp[:], in_=A[:, j, :], identity=ident_sb[:])
                nc.scalar.copy(out=Bs[:, j, :], in_=Bp[:])
                sd = ptmp.tile([128, nheads], dt, space="PSUM", tag="sd")
                nc.tensor.matmul(out=sd[:], lhsT=Bs[:, j, :], rhs=sdw(w), start=True, stop=True)
                nc.vector.tensor_tensor(
                    out=sc[:, j * nheads:(j + 1) * nheads], in0=sd[:],
                    in1=G[:, j, hw:hw + nheads], op=mybir.AluOpType.add)
            nc.scalar.activation(out=sc[:], in_=sc[:],
                                 func=mybir.ActivationFunctionType.Lrelu, alpha=NEG)
            nc.scalar.activation(out=sc[:], in_=sc[:], func=mybir.ActivationFunctionType.Exp)
            dn = pacc.tile([128, nheads], dt, space="PSUM", tag="dn")
            acc = pacc.tile([128, hw], dt, space="PSUM", tag="acc")
            vals = pool.tile([128, CH, hw], dt, tag="vals")
            for j in range(CH):
                nc.tensor.matmul(out=dn[:], lhsT=A[:, j, :], rhs=sc[:, j * nheads:(j + 1) * nheads],
                                 start=(j == 0), stop=(j == CH - 1))
                for h in range(nheads):
                    nc.vector.tensor_tensor(
                        out=vals[:, j, h * chs:(h + 1) * chs],
                        in0=G[:, j, h * chs:(h + 1) * chs],
                        in1=sc[:, j * nheads + h:j * nheads + h + 1].to_broadcast([128, chs]),
                        op=mybir.AluOpType.mult)
                nc.tensor.matmul(out=acc[:], lhsT=A[:, j, :], rhs=vals[:, j, :],
                                 start=(j == 0), stop=(j == CH - 1))
            rcp = pool.tile([128, nheads], dt, tag="rcp")
            nc.vector.reciprocal(out=rcp[:], in_=dn[:])
            o = pool.tile([128, hw], dt, tag="o")
            for h in range(nheads):
                nc.scalar.activation(out=o[:, h * chs:(h + 1) * chs], in_=acc[:, h * chs:(h + 1) * chs],
                                     func=mybir.ActivationFunctionType.Copy, scale=rcp[:, h:h + 1])
            nc.vector.tensor_tensor(out=o[:], in0=o[:], in1=bias_sb[:, :hw], op=mybir.AluOpType.add)
            if relu:
                nc.vector.tensor_scalar_max(o[:], o[:], 0.0)
            out_cb(w, o, ptmp)

    with tile.TileContext(nc) as tc:
        with tc.tile_pool(name="consts", bufs=1) as cpool:
            nc.sync.dma_start(out=iota_sb[:], in_=iota[:])
            nc.sync.dma_start(out=ident_sb[:], in_=ident[:])
            nc.sync.dma_start(out=isrc_sb[:], in_=isrc[:])
            nc.sync.dma_start(out=idstl_sb[:], in_=idstl[:])
            nc.sync.dma_start(out=dstlf_sb[:], in_=dstlf[:])
            nc.sync.dma_start(out=iself_sb[:], in_=iselft[:])
            w1_sb = cpool.tile([IN, 264], dt)
            nc.sync.dma_start(out=w1_sb[:], in_=W1p[:])
            b1_sb = cpool.tile([128, 256], dt)
            nc.sync.dma_start(out=b1_sb[:], in_=b1b[:])
            b2_sb = cpool.tile([128, 256], dt)
            nc.sync.dma_start(out=b2_sb[:], in_=b2b[:])
            bml_sb = cpool.tile([128, 64], dt)
            nc.sync.dma_start(out=bml_sb[:], in_=bmlb[:])
            we_sb = cpool.tile([128, 64], dt)
            nc.sync.dma_start(out=we_sb[:], in_=wedgeb[:])
            wfc_sb = cpool.tile([33, 64], dt)
            nc.sync.dma_start(out=wfc_sb[:], in_=Wfc1[:])
            wnode_sb = cpool.tile([65, 64], dt)
            nc.sync.dma_start(out=wnode_sb[:], in_=Wnode1[:])
            eps_sb = cpool.tile([128, W * LAT], dt)
            nc.sync.dma_start(out=eps_sb[:], in_=epst[:])
            w2_sb = cpool.tile([128, 2, 264], dt)
            nc.sync.dma_start(out=w2_sb[:], in_=W2p[:].rearrange("(k p) n -> p k n", p=128))
            wml_sb = cpool.tile([128, 2, 68], dt)
            nc.sync.dma_start(out=wml_sb[:], in_=Wml[:].rearrange("(k p) n -> p k n", p=128))

            # ---- L1 projection: full T1
            with tc.tile_pool(name="p1", bufs=3) as pool, \
                 tc.tile_pool(name="ps1", bufs=2, space="PSUM") as psum:
                for b in range(NWIN):
                    xt = pool.tile([IN, 128], dt, tag="xt")
                    nc.sync.dma_start(out=xt[:], in_=xT[:, b * 128:(b + 1) * 128])
                    pp = psum.tile([128, 264], dt, space="PSUM", tag="pp")
                    nc.tensor.matmul(out=pp[:], lhsT=xt[:], rhs=w1_sb[:], start=True, stop=True)
                    t = pool.tile([128, 264], dt, tag="t")
                    nc.scalar.copy(out=t[:], in_=pp[:])
                    nc.sync.dma_start(out=T1[b * 128:(b + 1) * 128, :], in_=t[:])

            # ---- own-window s_dst1 via self-row gather
            with tc.tile_pool(name="p1b", bufs=3) as pool:
                for w in range(W):
                    g = pool.tile([128, 264], dt, tag="gs")
                    nc.gpsimd.indirect_dma_start(
                        out=g[:], out_offset=None, in_=T1[:],
                        in_offset=bass.IndirectOffsetOnAxis(ap=iself_sb[:, w:w + 1], axis=0))
                    nc.vector.tensor_copy(out=sdst_sb[:, w * 4:(w + 1) * 4], in_=g[:, 260:264])

            # ---- L1 aggregation
            with tc.tile_pool(name="p2", bufs=2) as pool, \
                 tc.tile_pool(name="pa2", bufs=2, space="PSUM") as pacc, \
                 tc.tile_pool(name="pt2", bufs=2, space="PSUM") as ptmp:
                def out1(w, o, ptmp):
                    for half in range(2):
                        tp = ptmp.tile([128, 128], dt, space="PSUM", tag="Bp")
                        nc.tensor.transpose(out=tp[:], in_=o[:, half * 128:(half + 1) * 128],
                                            identity=ident_sb[:])
                        nc.scalar.copy(out=oT[:, half * SH + w * 128:half * SH + (w + 1) * 128],
                                       in_=tp[:])
                agg_layer(pool, pacc, ptmp, T1, lambda w: sdst_sb[:, w * 4:(w + 1) * 4],
                          4, 64, b1_sb, True, out1)

            # ---- L2 projection (own shard) + AllGather
            with tc.tile_pool(name="p3", bufs=3) as pool, \
                 tc.tile_pool(name="ps3", bufs=2, space="PSUM") as psum:
                for b in range(W):
                    pp = psum.tile([128, 264], dt, space="PSUM", tag="pp2")
                    for k in range(2):
                        nc.tensor.matmul(out=pp[:], lhsT=oT[:, k * SH + b * 128:k * SH + (b + 1) * 128],
                                         rhs=w2_sb[:, k, :], start=(k == 0), stop=(k == 1))
                    t = pool.tile([128, 264], dt, tag="t2")
                    nc.scalar.copy(out=t[:], in_=pp[:])
                    nc.sync.dma_start(out=sh2[b * 128:(b + 1) * 128, :], in_=t[:])
                    nc.vector.tensor_copy(out=sdst_sb[:, b * 4:(b + 1) * 4], in_=t[:, 260:264])
            nc.gpsimd.collective_compute("AllGather", mybir.AluOpType.bypass,
                                         replica_groups=RG, ins=[sh2[:]], outs=[T2[:]])

            # ---- L2 aggregation
            with tc.tile_pool(name="p4", bufs=2) as pool, \
                 tc.tile_pool(name="pa4", bufs=2, space="PSUM") as pacc, \
                 tc.tile_pool(name="pt4", bufs=2, space="PSUM") as ptmp:
                def out2(w, o, ptmp):
                    for half in range(2):
                        tp = ptmp.tile([128, 128], dt, space="PSUM", tag="Bp")
                        nc.tensor.transpose(out=tp[:], in_=o[:, half * 128:(half + 1) * 128],
                                            identity=ident_sb[:])
                        nc.scalar.copy(out=oT[:, half * SH + w * 128:half * SH + (w + 1) * 128],
                                       in_=tp[:])
                agg_layer(pool, pacc, ptmp, T2, lambda w: sdst_sb[:, w * 4:(w + 1) * 4],
                          4, 64, b2_sb, True, out2)

            # ---- ml projection + AllGather
            with tc.tile_pool(name="p5", bufs=3) as pool, \
                 tc.tile_pool(name="ps5", bufs=2, space="PSUM") as psum:
                for b in range(W):
                    pp = psum.tile([128, 68], dt, space="PSUM", tag="ppm")
                    for k in range(2):
                        nc.tensor.matmul(out=pp[:], lhsT=oT[:, k * SH + b * 128:k * SH + (b + 1) * 128],
                                         rhs=wml_sb[:, k, :], start=(k == 0), stop=(k == 1))
                    t = pool.tile([128, 68], dt, tag="tm")
                    nc.scalar.copy(out=t[:], in_=pp[:])
                    nc.sync.dma_start(out=shml[b * 128:(b + 1) * 128, :], in_=t[:])
                    nc.vector.tensor_copy(out=sdst_sb[:, b * 4:b * 4 + 2], in_=t[:, 66:68])
            nc.gpsimd.collective_compute("AllGather", mybir.AluOpType.bypass,
                                         replica_groups=RG, ins=[shml[:]], outs=[Tml[:]])

            # ---- ml aggregation + reparam
            with tc.tile_pool(name="p6", bufs=2) as pool, \
                 tc.tile_pool(name="pa6", bufs=2, space="PSUM") as pacc, \
                 tc.tile_pool(name="pt6", bufs=2, space="PSUM") as ptmp:
                def out3(w, o, ptmp):
                    nc.sync.dma_start(out=mu_o[:, w * LAT:(w + 1) * LAT], in_=o[:, 0:LAT])
                    nc.sync.dma_start(out=lv_o[:, w * LAT:(w + 1) * LAT], in_=o[:, LAT:2 * LAT])
                    elv = pool.tile([128, LAT], dt, tag="elv")
                    nc.scalar.activation(out=elv[:], in_=o[:, LAT:2 * LAT],
                                         func=mybir.ActivationFunctionType.Exp, scale=0.5)
                    z = pool.tile([128, LAT], dt, tag="z")
                    nc.vector.tensor_tensor(out=z[:], in0=elv[:],
                                            in1=eps_sb[:, w * LAT:(w + 1) * LAT],
                                            op=mybir.AluOpType.mult)
                    nc.vector.tensor_tensor(out=z[:], in0=z[:], in1=o[:, 0:LAT],
                                            op=mybir.AluOpType.add)
                    nc.sync.dma_start(out=z_o[:, w * LAT:(w + 1) * LAT], in_=z[:])
                    zp = ptmp.tile([128, 128], dt, space="PSUM", tag="Bp")
                    nc.tensor.transpose(out=zp[:LAT, :], in_=z[:], identity=ident_sb[:])
                    nc.scalar.copy(out=zT_sb[0:LAT, w * 128:(w + 1) * 128], in_=zp[:LAT, :])
                agg_layer(pool, pacc, ptmp, Tml, lambda w: sdst_sb[:, w * 4:w * 4 + 2],
                          2, 32, bml_sb, False, out3)

            # ---- decoder
            with tc.tile_pool(name="p7", bufs=3) as pool, \
                 tc.tile_pool(name="ps7", bufs=2, space="PSUM") as psum:
                nc.gpsimd.memset(zT_sb[LAT:LAT + 1, :], 1.0)
                nc.gpsimd.memset(zdT_sb[64:65, :], 1.0)
                for b in range(W):
                    zdp = psum.tile([64, 128], dt, space="PSUM", tag="zdp")
                    nc.tensor.matmul(out=zdp[:], lhsT=wfc_sb[:],
                                     rhs=zT_sb[0:33, b * 128:(b + 1) * 128], start=True, stop=True)
                    nc.scalar.activation(out=zdT_sb[0:64, b * 128:(b + 1) * 128], in_=zdp[:],
                                         func=mybir.ActivationFunctionType.Relu)
                for b in range(W):
                    nfp = psum.tile([128, 64], dt, space="PSUM", tag="nfp")
                    nc.tensor.matmul(out=nfp[:], lhsT=zdT_sb[0:65, b * 128:(b + 1) * 128],
                                     rhs=wnode_sb[:], start=True, stop=True)
                    nf = pool.tile([128, 64], dt, tag="nf")
                    nc.scalar.copy(out=nf[:], in_=nfp[:])
                    nc.sync.dma_start(out=nf_o[:, b * 64:(b + 1) * 64], in_=nf[:])
                    zwp = psum.tile([128, 128], dt, space="PSUM", tag="zwp")
                    nc.tensor.transpose(out=zwp[:, :64], in_=zdT_sb[0:64, b * 128:(b + 1) * 128],
                                        identity=ident_sb[0:64, 0:64])
                    zdw = pool.tile([128, 64], dt, tag="zdw")
                    nc.scalar.copy(out=zdw[:], in_=zwp[:, :64])
                    nc.sync.dma_start(out=Tzd[b * 128:(b + 1) * 128, :], in_=zdw[:])
                    uw = pool.tile([128, 64], dt, tag="uw")
                    nc.vector.tensor_tensor(out=uw[:], in0=zdw[:], in1=we_sb[:],
                                            op=mybir.AluOpType.mult)
                    nc.sync.dma_start(out=shu[b * 128:(b + 1) * 128, :], in_=uw[:])
            nc.gpsimd.collective_compute("AllGather", mybir.AluOpType.bypass,
                                         replica_groups=RG, ins=[shu[:]], outs=[Tu[:]])

            # ---- edge scores
            with tc.tile_pool(name="p8", bufs=4) as pool:
                sc_all = pool.tile([128, NCHUNK], F32, tag="scall")
                for w in range(W):
                    for j in range(CH):
                        col = w * CH + j
                        gu = pool.tile([128, 64], F32, tag="gu")
                        nc.gpsimd.indirect_dma_start(
                            out=gu[:], out_offset=None, in_=Tu[:],
                            in_offset=bass.IndirectOffsetOnAxis(ap=isrc_sb[:, col:col + 1], axis=0))
                        gz = pool.tile([128, 64], F32, tag="gz")
                        nc.gpsimd.indirect_dma_start(
                            out=gz[:], out_offset=None, in_=Tzd[:],
                            in_offset=bass.IndirectOffsetOnAxis(ap=idstl_sb[:, col:col + 1], axis=0))
                        pr = pool.tile([128, 64], F32, tag="pr")
                        nc.vector.tensor_tensor(out=pr[:], in0=gu[:], in1=gz[:],
                                                op=mybir.AluOpType.mult)
                        nc.vector.tensor_reduce(out=sc_all[:, col:col + 1], in_=pr[:],
                                                axis=mybir.AxisListType.X, op=mybir.AluOpType.add)
                nc.vector.tensor_scalar_add(sc_all[:], sc_all[:], float(b_edge_val))
                nc.sync.dma_start(out=sc_o[:], in_=sc_all[:])

    stack.close()
    nc.compile()
    return nc


_CACHE = {}


def kernel(**inputs):
    x = np.asarray(inputs["x"], np.float32)
    edge_index = np.asarray(inputs["edge_index"])
    eps = np.asarray(inputs["eps"], np.float32)

    pp = _prep(edge_index)
    pnode, inv, used = pp["pnode"], pp["inv"], pp["used"]

    W1p = _fold(np.asarray(inputs["W1"], np.float32), np.asarray(inputs["a_src1"], np.float32),
                np.asarray(inputs["a_dst1"], np.float32))
    W2p = _fold(np.asarray(inputs["W2"], np.float32), np.asarray(inputs["a_src2"], np.float32),
                np.asarray(inputs["a_dst2"], np.float32))
    Wmu = np.asarray(inputs["Wmu"], np.float32)
    Wlv = np.asarray(inputs["Wlv"], np.float32)
    Wml = np.concatenate([
        Wmu, Wlv,
        Wmu @ np.asarray(inputs["a_src_mu"], np.float32)[0][:, None],
        Wlv @ np.asarray(inputs["a_src_lv"], np.float32)[0][:, None],
        Wmu @ np.asarray(inputs["a_dst_mu"], np.float32)[0][:, None],
        Wlv @ np.asarray(inputs["a_dst_lv"], np.float32)[0][:, None]], axis=1).astype(np.float32)
    Wfc1 = np.concatenate([np.asarray(inputs["W_fc"], np.float32),
                           np.asarray(inputs["b_fc"], np.float32)[None, :]], axis=0)
    Wnode1 = np.concatenate([np.asarray(inputs["W_node"], np.float32),
                             np.asarray(inputs["b_node"], np.float32)[None, :]], axis=0)
    b_edge = float(np.asarray(inputs["b_edge"], np.float32).reshape(-1)[0])

    xp = np.zeros((NP, IN), np.float32)
    xp[pnode] = x
    xT = np.ascontiguousarray(xp.T)

    b1b = np.broadcast_to(np.asarray(inputs["b1"], np.float32)[None, :], (128, 256)).copy()
    b2b = np.broadcast_to(np.asarray(inputs["b2"], np.float32)[None, :], (128, 256)).copy()
    bml = np.concatenate([np.asarray(inputs["b_mu"], np.float32),
                          np.asarray(inputs["b_lv"], np.float32)])
    bmlb = np.broadcast_to(bml[None, :], (128, 64)).copy()
    wedgeb = np.broadcast_to(np.asarray(inputs["W_edge"], np.float32)[:, 0][None, :], (128, 64)).copy()
    iota = np.broadcast_to(np.arange(128, dtype=np.float32)[None, :], (128, 128)).copy()
    ident = np.eye(128, dtype=np.float32)

    eps_p = np.zeros((NP, LAT), np.float32)
    eps_p[pnode] = eps

    if "nc" not in _CACHE:
        _CACHE["nc"] = _build(b_edge)
    ncb = _CACHE["nc"]

    in_maps = []
    for c in range(NC):
        rows = np.arange(c * SH, (c + 1) * SH)
        epst = eps_p[rows].reshape(W, 128, LAT).transpose(1, 0, 2).reshape(128, W * LAT)
        iself = np.ascontiguousarray(
            (c * SH + np.arange(SH, dtype=np.int32)).reshape(W, 128).T)
        in_maps.append({
            "xT": xT, "W1p": W1p, "W2p": W2p, "Wml": Wml, "Wfc1": Wfc1,
            "Wnode1": Wnode1, "b1b": b1b, "b2b": b2b, "bmlb": bmlb,
            "wedgeb": wedgeb, "iota": iota, "ident": ident,
            "isrc": pp["idx_src"][c], "idstl": pp["idx_dstl"][c],
            "dstlf": pp["dstl_f"][c], "iselft": iself,
            "epst": np.ascontiguousarray(epst),
        })

    res = run_bass_kernel_spmd(ncb, in_maps, core_ids=list(range(NC)))

    mu = np.zeros((N, LAT), np.float32)
    lv = np.zeros((N, LAT), np.float32)
    z = np.zeros((N, LAT), np.float32)
    nf = np.zeros((N, IN), np.float32)
    sc = np.zeros(E, np.float32)
    for c in range(NC):
        r = res.results[c]
        rows = np.arange(c * SH, (c + 1) * SH)
        m = used[rows]
        orig = inv[rows][m]
        for name, arr, d in (("mu_o", mu, LAT), ("lv_o", lv, LAT),
                             ("z_o", z, LAT), ("nf_o", nf, IN)):
            v = np.asarray(r[name]).reshape(128, W, d).transpose(1, 0, 2).reshape(SH, d)
            arr[orig] = v[m]
        scv = np.asarray(r["sc_o"])
        ep = pp["epos"][c]
        valid = ep >= 0
        sc[ep[valid]] = scv[valid]

    return (mu, lv, z, nf, sc)
